# revision 49
# baseline (speedup 1.0000x reference)
"""Trainium2 Bass kernel: channel-attention MultiHeadAttention block.

Full (unsharded) inputs in, full output out. Data-parallel over batch B
across 8 NeuronCores (1 batch each) with one tiny AllReduce for BatchNorm
batch statistics.

Algebraic structure (per core, batch b). All projections are folded through
the 65x65 Gram matrix so the PE never materializes Q/K/V:

  qe/ke/ve  [65, 4096]   raw inputs + ones row (bias fold), bf16
  G         [65, 65]     = qe @ ke^T     (32 small accumulating matmuls on
                           host-pre-transposed qT/kT chunks)
  scores    [512, 512]   = wqe^T G wke   (tiny expansion: GT, H=G@wke, 4 mm)
  attn      [128c,512d]  exp(scores/64)/rowsum (ACT exp + DVE normalize)
  aT        [128d,512c]  XBAR DMA-transpose of attn (no PE)
  A2        [65, 512]    = wv_ext^T @ attn^T  (4 matmuls; wv_ext=[wv|bv])
  X[cc]     [128c',4096] = A2^T @ ve via stride-8 ve slices as stationary:
                           X[c', 512r+c] = attnout[c, 8c'+r]  (BN layout)
  BN stats  copy+sum (ACT) / square+sum (DVE) -> AllReduce -> alpha/beta
  BN+leaky  in-place ACT Lrelu(scale=alpha, bias=beta)
  w1+leaky  [512,512] conv (bf16), bias+leaky fused in ACT
  w2+bias   [64,512] conv -> y [64, 4096] f32
"""

import sys

if "/opt/trn_rl_repo" not in sys.path:
    sys.path.insert(0, "/opt/trn_rl_repo")

import numpy as np

import concourse.bacc as bacc
import concourse.mybir as mybir
import concourse.tile as tile
from concourse import bass_utils

B = 8
C = 64
CN = 512
HW = 4096
NCH = CN // 128  # 4 channel chunks
NS = HW // 512   # 8 free-dim slices
NMC = HW // 128  # 32 m-chunks for the Gram accumulation
EPS = 1e-4
SLOPE = 0.01
INV_SCALE = 1.0 / 64.0      # 1/sqrt(HW)
INV_BHW = 1.0 / (B * HW)    # BN divisor

F32 = mybir.dt.float32
F32R = mybir.dt.float32r
BF16 = mybir.dt.bfloat16
AF = mybir.ActivationFunctionType
ALU = mybir.AluOpType
AX = mybir.AxisListType
RG = [[0, 1, 2, 3, 4, 5, 6, 7]]

# wmisc f32 column layout
WM_ID = 0          # eye(128)
WM_B1 = 128        # [128, 4]
WM_B2 = 132        # [64, 1]
WM_BNG = 136       # [128, 4]
WM_BNB = 140       # [128, 4]
WM_SEL = 144       # [64, 8] AllGather reduce selector: tile(eye(8), (8,1))
WM_N = 152

# wpk16 bf16 column layout
W16_W1 = 0         # 4 x [128, 512]
W16_W2 = 2048      # 4 x [128, 64]
W16_WVT = 2304     # 4 x [128, 65]
W16_N = 2564


def _body(tc, nc, d, dbg=None):
    with (
        tc.tile_pool(name="consts", bufs=1) as consts,
        tc.tile_pool(name="small", bufs=1) as small,
    ):
        # ---- inputs + weights. qt/kt split into quarters round-robined
        # over the 3 DMA queues (sync=SP, scalar=ACT, gpsimd) so the Gram
        # matmuls can start as soon as the first chunks land.
        qt = consts.tile([128, 32 * 65], BF16, name="qt", tag="qt")
        kt = consts.tile([128, 32 * 65], BF16, name="kt", tag="kt")
        vsb = consts.tile([65, HW], BF16, name="vsb", tag="vsb")
        wqek = consts.tile([65, 1024], F32R, name="wqek", tag="wqek")
        wmisc = consts.tile([128, WM_N], F32R, name="wmisc", tag="wmisc")
        wpk16 = consts.tile([128, W16_N], BF16, name="wpk16", tag="wpk16")
        # chunk sizes ramp up so the Gram matmuls start ASAP
        QCH = [130, 130, 260, 260, 390, 390, 260, 260]
        off = 0
        for g, w in enumerate(QCH):
            qsl = slice(off, off + w)
            off += w
            qeng = (nc.sync, nc.gpsimd, nc.scalar)[g % 3]
            keng = (nc.scalar, nc.sync, nc.gpsimd)[g % 3]
            qeng.dma_start(qt[:, qsl], d["qt"][:, qsl])
            keng.dma_start(kt[:, qsl], d["kt"][:, qsl])
            if g == 2:
                # wqek early: the H matmul needs it right after the Gram
                nc.gpsimd.dma_start(wqek[:], d["wqek"][:])
        nc.gpsimd.dma_start(wmisc[:], d["wmisc"][:])
        nc.scalar.dma_start(wpk16[:, W16_WVT:W16_N],
                            d["wpk16"][:, W16_WVT:W16_N])
        nc.gpsimd.dma_start(vsb[:], d["v"][:])
        nc.sync.dma_start(wpk16[:, 0:W16_WVT], d["wpk16"][:, 0:W16_WVT])

        wqe = wqek[0:65, 0:512]
        wke = wqek[0:65, 512:1024]
        ident = wmisc[:, WM_ID:WM_ID + 128]
        b1sb = wmisc[:, WM_B1:WM_B1 + 4].bitcast(F32)
        b2sb = wmisc[0:64, WM_B2:WM_B2 + 1].bitcast(F32)
        bngsb = wmisc[:, WM_BNG:WM_BNG + 4].bitcast(F32)
        bnbsb = wmisc[:, WM_BNB:WM_BNB + 4].bitcast(F32)
        selsb = wmisc[0:64, WM_SEL:WM_SEL + 8]
        w1sb = [wpk16[:, W16_W1 + 512 * cc:W16_W1 + 512 * (cc + 1)]
                for cc in range(NCH)]
        w2sb = [wpk16[:, W16_W2 + 64 * oc:W16_W2 + 64 * (oc + 1)]
                for oc in range(NCH)]
        wvt = [wpk16[:, W16_WVT + 65 * j:W16_WVT + 65 * (j + 1)]
               for j in range(NCH)]

        # ACT Exp table preload while DMAs are in flight (one table resident
        # at a time; the Lrelu set is loaded behind the stats phase below)
        epsb = small.tile([128, 1], F32, name="epsb", tag="epsb")
        nc.vector.memset(epsb[:], EPS)
        dummy = small.tile([128, 1], F32, name="dummy", tag="dummy")
        nc.scalar.activation(dummy[:], epsb[:], AF.Exp, bias=0.0, scale=1.0)

        alpha = small.tile([128, 4], F32, name="alpha", tag="alpha")
        beta = small.tile([128, 4], F32, name="beta", tag="beta")

        X = [consts.tile([128, HW], BF16, name=f"X{cc}", tag=f"X{cc}")
             for cc in range(NCH)]
        aTall = consts.tile([128, CN * NCH], BF16, name="aTall", tag="aTall")
        A2sb = consts.tile([65, CN], BF16, name="A2sb", tag="A2sb")

        # ================= phase 1: Gram -> scores -> attn -> A2 =========
        with tc.tile_pool(name="sc32", bufs=1) as sc32:
            with tc.tile_pool(name="ps_g", bufs=1, space="PSUM") as psg:
                # GT = ke @ qe^T [65cj, 65ci] (Gram, swapped: no transpose)
                GTps = psg.tile([65, 65], F32, name="GTps", tag="GTps")
                for j in range(NMC):
                    nc.tensor.matmul(GTps[:], kt[:, 65 * j:65 * (j + 1)],
                                     qt[:, 65 * j:65 * (j + 1)],
                                     start=(j == 0), stop=(j == NMC - 1))
                GTsb = sc32.tile([65, 65], F32R, name="GTsb", tag="GTsb")
                nc.vector.tensor_copy(GTsb[:], GTps[:])
                # H = G @ wke = GT^T @ wke  [65ci, 512]
                Hps = psg.tile([65, 512], F32, name="Hps", tag="Hps")
                nc.tensor.matmul(Hps[:], GTsb[:], wke, start=True, stop=True)
                Hsb = sc32.tile([65, 512], F32R, name="Hsb", tag="Hsb")
                nc.scalar.copy(Hsb[:], Hps[:])

            # scores chunks + exp + normalize
            attnb = sc32.tile([128, CN * NCH], F32R, name="attnb",
                              tag="attnb")
            rowsum = small.tile([128, 4], F32, name="rowsum", tag="rowsum")
            recip = small.tile([128, 4], F32, name="recip", tag="recip")
            with tc.tile_pool(name="ps_sc", bufs=1, space="PSUM") as pssc:
                sc = [pssc.tile([128, 512], F32, name=f"sc{cc}",
                                tag=f"sc{cc}") for cc in range(NCH)]
                for cc in range(NCH):
                    nc.tensor.matmul(sc[cc][:],
                                     wqe[:, 128 * cc:128 * (cc + 1)],
                                     Hsb[:], start=True, stop=True)
                    nc.scalar.activation(attnb[:, 512 * cc:512 * (cc + 1)],
                                         sc[cc][:], AF.Exp,
                                         bias=0.0, scale=INV_SCALE,
                                         accum_out=rowsum[:, cc:cc + 1])
                # Lrelu table load hidden behind X2/stats work; gated on
                # rowsum so the scheduler cannot hoist it before the exps
                nc.scalar.activation(dummy[:], rowsum[:, 3:4], AF.Lrelu,
                                     bias=0.0, scale=1.0, alpha=SLOPE)
                nc.vector.reciprocal(recip[:], rowsum[:])
                # normalize on ACT (same queue as exp -> no cross-engine hop
                # before the transposes)
                for cc in range(NCH):
                    nc.scalar.activation(
                        attnb[:, 512 * cc:512 * (cc + 1)],
                        attnb[:, 512 * cc:512 * (cc + 1)],
                        AF.Copy, bias=0.0, scale=recip[:, cc:cc + 1])

            # aT via PE transposes: block j holds d in [128j, 128j+128)
            with (
                tc.tile_pool(name="tps", bufs=2, space="PSUM") as tps,
                tc.tile_pool(name="ps_a2", bufs=1, space="PSUM") as psa2,
            ):
                for j in range(NCH):
                    tp = tps.tile([128, 512], F32R, name="tp", tag="tp")
                    for cc in range(NCH):
                        nc.tensor.transpose(
                            tp[:, 128 * cc:128 * (cc + 1)],
                            attnb[:, 512 * cc + 128 * j:
                                   512 * cc + 128 * (j + 1)],
                            ident)
                    if j % 2 == 0:
                        nc.vector.tensor_copy(
                            aTall[:, 512 * j:512 * (j + 1)], tp[:])
                    else:
                        nc.scalar.copy(
                            aTall[:, 512 * j:512 * (j + 1)], tp[:])

                # A2 = wv_ext^T @ attn^T  [65, 512]
                A2ps = psa2.tile([65, 512], F32, name="A2ps", tag="A2ps")
                for j in range(NCH):
                    nc.tensor.matmul(A2ps[:], wvt[j],
                                     aTall[:, 512 * j:512 * (j + 1)],
                                     start=(j == 0), stop=(j == NCH - 1))
                nc.scalar.copy(A2sb[:], A2ps[:])

            if dbg is not None:
                nc.sync.dma_start(dbg["attnb"][:], attnb[:])

        if dbg is not None:
            nc.sync.dma_start(dbg["aTall"][:], aTall[:])
            nc.sync.dma_start(dbg["A2sb"][:], A2sb[:])

        # ============ phase 2: X (BN layout) + stats + AllReduce =========
        vre = vsb[:, :].rearrange("ci (cp r) -> ci r cp", r=8)
        with (
            tc.tile_pool(name="stp", bufs=1) as stp,
            tc.tile_pool(name="junkp", bufs=2) as junkp,
            tc.tile_pool(name="cdram", bufs=1, space="DRAM") as cdram,
            tc.tile_pool(name="ps_x", bufs=3, space="PSUM") as psx,
        ):
            pssum = stp.tile([128, 16], F32, name="pssum", tag="pssum")
            pssq = stp.tile([128, 16], F32, name="pssq", tag="pssq")
            red = stp.tile([128, 8], F32, name="red", tag="red")
            for cc in range(NCH):
                for rp in range(4):
                    xt = psx.tile([128, 1024], F32, name="xt", tag="xt")
                    for h in range(2):
                        r = 2 * rp + h
                        nc.tensor.matmul(
                            xt[:, 512 * h:512 * (h + 1)],
                            vre[:, r, 128 * cc:128 * (cc + 1)],
                            A2sb[:], start=True, stop=True)
                    slot = 4 * cc + rp
                    xsl = slice(1024 * rp, 1024 * (rp + 1))
                    nc.scalar.activation(X[cc][:, xsl], xt[:], AF.Copy,
                                         accum_out=pssum[:, slot:slot + 1])
                    junk = junkp.tile([128, 1024], BF16, name="junk",
                                      tag="junk")
                    nc.vector.scalar_tensor_tensor(
                        junk[:], X[cc][:, xsl], 1.0, X[cc][:, xsl],
                        op0=ALU.bypass, op1=ALU.mult,
                        accum_out=pssq[:, slot:slot + 1])
            for cc in range(NCH):
                nc.vector.reduce_sum(red[:, 2 * cc:2 * cc + 1],
                                     pssum[:, 4 * cc:4 * (cc + 1)], axis=AX.X)
                nc.vector.reduce_sum(red[:, 2 * cc + 1:2 * cc + 2],
                                     pssq[:, 4 * cc:4 * (cc + 1)], axis=AX.X)

            # BN-stats exchange. Send red transposed [8, 128] so the cin
            # write and the post-AllGather readback both use 512B-contiguous
            # descriptors; reduce over cores with a single selector matmul
            # that lands directly in [128, 8] per-channel orientation.
            with tc.tile_pool(name="arps", bufs=1, space="PSUM") as arps:
                redR = stp.tile([128, 8], F32R, name="redR", tag="redR")
                nc.vector.tensor_copy(redR[:], red[:])
                redTps = arps.tile([8, 128], F32R, name="redTps",
                                   tag="redTps")
                nc.tensor.transpose(redTps[:], redR[:], ident)
                redT = stp.tile([8, 128], F32R, name="redT", tag="redT")
                nc.vector.tensor_copy(redT[:], redTps[:])
                cin = cdram.tile([8, 128], F32R, name="cin", tag="cin")
                cout = cdram.tile([64, 128], F32R, name="cout", tag="cout")
                nc.sync.dma_start(cin[:], redT[:])
                nc.gpsimd.collective_compute(
                    "AllGather", ALU.bypass, replica_groups=RG,
                    ins=[cin.opt()], outs=[cout.opt()])
                gath = stp.tile([64, 128], F32R, name="gath", tag="gath")
                nc.sync.dma_start(gath[:], cout[:])
                arp = arps.tile([128, 8], F32, name="arp", tag="arp")
                nc.tensor.matmul(arp[:], gath[:], selsb,
                                 start=True, stop=True)

                # BN affine params; rstd = (var+eps)^-1/2 via bit-trick
                # seed + 1 Newton step, all on DVE (no ACT table switch)
                mean = stp.tile([128, 4], F32, name="mean", tag="mean")
                var = stp.tile([128, 4], F32, name="var", tag="var")
                rstd = stp.tile([128, 4], F32, name="rstd", tag="rstd")
                tmp = stp.tile([128, 4], F32, name="tmpb", tag="tmpb")
                mv = stp.tile([128, 8], F32, name="mv", tag="mv")
                nc.vector.tensor_scalar_mul(mv[:], arp[:], INV_BHW)
                mvr = mv[:, :].rearrange("p (c two) -> p two c", two=2)
                nc.vector.tensor_mul(tmp[:], mvr[:, 0, :], mvr[:, 0, :])
                nc.vector.tensor_sub(var[:], mvr[:, 1, :], tmp[:])
                nc.vector.tensor_scalar_add(var[:], var[:], EPS)
                I32 = mybir.dt.int32
                nc.vector.tensor_scalar(
                    out=tmp[:].bitcast(I32), in0=var[:].bitcast(I32),
                    scalar1=1, scalar2=None, op0=ALU.arith_shift_right)
                nc.vector.tensor_scalar(
                    out=rstd[:].bitcast(I32), in0=tmp[:].bitcast(I32),
                    scalar1=-1, scalar2=0x5f3759df, op0=ALU.mult,
                    op1=ALU.add)
                for _ in range(1):
                    nc.vector.tensor_mul(tmp[:], rstd[:], rstd[:])
                    nc.vector.tensor_mul(tmp[:], tmp[:], var[:])
                    nc.vector.tensor_scalar(
                        out=tmp[:], in0=tmp[:], scalar1=-0.5, scalar2=1.5,
                        op0=ALU.mult, op1=ALU.add)
                    nc.vector.tensor_mul(rstd[:], rstd[:], tmp[:])
                nc.vector.tensor_mul(alpha[:], bngsb[:], rstd[:])
                nc.vector.tensor_mul(tmp[:], mvr[:, 0, :], alpha[:])
                nc.vector.tensor_sub(beta[:], bnbsb[:], tmp[:])
                if dbg is not None:
                    nc.sync.dma_start(dbg["rstd"][:], rstd[:])

            if dbg is not None:
                for cc in range(NCH):
                    nc.sync.dma_start(dbg[f"X{cc}"][:], X[cc][:])
                nc.sync.dma_start(dbg["red"][:], red[:])
                nc.sync.dma_start(dbg["ar"][:], ar[:])
                nc.sync.dma_start(dbg["ab"][:, 0:4], alpha[:])
                nc.sync.dma_start(dbg["ab"][:, 4:8], beta[:])

        # ========= phase 3: BN+leaky (interleaved) -> w1 -> w2 -> y ======
        # BN + leaky in place: slice 0 on ACT (fastest start), slice 1 on
        # DVE (frees ACT for the first y1 ops); the remaining ms-pair
        # chunks are emitted inside the ms loop (queues are in-order, so
        # emitting all BN up front would starve the early y1 ops).
        with (
            tc.tile_pool(name="bnj", bufs=2) as bnj,
            tc.tile_pool(name="y2", bufs=2) as y2p,
            tc.tile_pool(name="outb", bufs=1) as outp,
            tc.tile_pool(name="wps", bufs=4, space="PSUM") as wps,
            tc.tile_pool(name="w2ps", bufs=2, space="PSUM") as w2ps,
        ):
            def bn_act(cc, xsl):
                nc.scalar.activation(X[cc][:, xsl], X[cc][:, xsl],
                                     AF.Lrelu,
                                     bias=beta[:, cc:cc + 1],
                                     scale=alpha[:, cc:cc + 1],
                                     alpha=SLOPE)

            def bn_dve(cc, xsl):
                w = xsl.stop - xsl.start
                bt = bnj.tile([128, 1024], BF16, name="bt", tag="bt")
                nc.vector.tensor_scalar(
                    out=bt[:, 0:w], in0=X[cc][:, xsl],
                    scalar1=alpha[:, cc:cc + 1],
                    scalar2=beta[:, cc:cc + 1],
                    op0=ALU.mult, op1=ALU.add)
                nc.vector.scalar_tensor_tensor(
                    X[cc][:, xsl], bt[:, 0:w], SLOPE, bt[:, 0:w],
                    op0=ALU.mult, op1=ALU.max)

            for cc in range(NCH):
                bn_act(cc, slice(0, 512))
            for cc in range(NCH):
                bn_dve(cc, slice(512, 1024))

            osb = outp.tile([64, HW], F32, name="osb", tag="osb")
            for ms in range(NS):
                if ms in (1, 3, 5):
                    mp = (ms + 1) // 2
                    xsl = slice(1024 * mp, 1024 * (mp + 1))
                    for cc in (0, 1):
                        bn_act(cc, xsl)
                    for cc in (2, 3):
                        bn_dve(cc, xsl)
                ssl = slice(512 * ms, 512 * (ms + 1))
                y2t = []
                for oc in range(NCH):
                    wp = wps.tile([128, 512], F32, name="wp", tag="wp")
                    for cc in range(NCH):
                        nc.tensor.matmul(
                            wp[:], w1sb[cc][:, 128 * oc:128 * (oc + 1)],
                            X[cc][:, ssl], start=(cc == 0), stop=(cc == 3))
                    yt = y2p.tile([128, 512], BF16, name=f"y2_{oc}",
                                  tag=f"y2_{oc}")
                    nc.scalar.activation(yt[:], wp[:], AF.Lrelu,
                                         bias=b1sb[:, oc:oc + 1],
                                         scale=1.0, alpha=SLOPE)
                    y2t.append(yt)
                fp = w2ps.tile([64, 512], F32, name="fp", tag="fp")
                for oc in range(NCH):
                    nc.tensor.matmul(fp[:], w2sb[oc], y2t[oc][:],
                                     start=(oc == 0), stop=(oc == 3))
                nc.vector.tensor_scalar_add(osb[:, ssl], fp[:],
                                            b2sb[:, 0:1])
                nc.sync.dma_start(d["y"][:, ssl], osb[:, ssl])


_NC_CACHE = {}


def _build(debug=False):
    key = ("dbg" if debug else "nc")
    if key in _NC_CACHE:
        return _NC_CACHE[key]
    nc = bacc.Bacc(trn_type="TRN2", target_bir_lowering=False, debug=False,
                   enable_asserts=False, num_devices=8)
    d = {}
    d["qt"] = nc.dram_tensor("qt", (128, 32 * 65), BF16,
                             kind="ExternalInput").ap()
    d["kt"] = nc.dram_tensor("kt", (128, 32 * 65), BF16,
                             kind="ExternalInput").ap()
    d["v"] = nc.dram_tensor("v", (65, HW), BF16, kind="ExternalInput").ap()
    d["wqek"] = nc.dram_tensor("wqek", (65, 1024), F32R,
                               kind="ExternalInput").ap()
    d["wmisc"] = nc.dram_tensor("wmisc", (128, WM_N), F32R,
                                kind="ExternalInput").ap()
    d["wpk16"] = nc.dram_tensor("wpk16", (128, W16_N), BF16,
                                kind="ExternalInput").ap()
    d["y"] = nc.dram_tensor("y", (64, HW), F32, kind="ExternalOutput").ap()

    dbg = None
    if debug:
        dbg = {}
        dbg["attnb"] = nc.dram_tensor("dbg_attnb", (128, 2048), F32R,
                                      kind="ExternalOutput").ap()
        dbg["rstd"] = nc.dram_tensor("dbg_rstd", (128, 4), F32,
                                     kind="ExternalOutput").ap()
        dbg["aTall"] = nc.dram_tensor("dbg_aTall", (128, 2048), BF16,
                                      kind="ExternalOutput").ap()
        dbg["A2sb"] = nc.dram_tensor("dbg_A2sb", (65, 512), BF16,
                                     kind="ExternalOutput").ap()
        for cc in range(NCH):
            dbg[f"X{cc}"] = nc.dram_tensor(f"dbg_X{cc}", (128, HW), BF16,
                                           kind="ExternalOutput").ap()
        dbg["red"] = nc.dram_tensor("dbg_red", (128, 8), F32,
                                    kind="ExternalOutput").ap()
        dbg["ar"] = nc.dram_tensor("dbg_ar", (128, 8), F32,
                                   kind="ExternalOutput").ap()
        dbg["ab"] = nc.dram_tensor("dbg_ab", (128, 8), F32,
                                   kind="ExternalOutput").ap()
    with tile.TileContext(nc) as tc:
        _body(tc, nc, d, dbg)
    nc.compile()
    _NC_CACHE[key] = nc
    return nc


def _to_bf16(a):
    import ml_dtypes
    return a.astype(ml_dtypes.bfloat16)


def _run(q, k, v, wq, bq, wk, bk, wv, bv, bn_g, bn_b, w1, b1, w2, b2,
         trace=False, tmpdir=None, debug=False):
    nc = _build(debug)
    f = np.float32
    ones = np.ones((1, HW), f)

    wqek = np.zeros((65, 1024), f)
    wqek[:, 0:512] = np.concatenate([wq.T, bq[None, :]], axis=0)
    wqek[:, 512:1024] = np.concatenate([wk.T, bk[None, :]], axis=0)
    wmisc = np.zeros((128, WM_N), f)
    wmisc[:, WM_ID:WM_ID + 128] = np.eye(128, dtype=f)
    wmisc[:, WM_B1:WM_B1 + 4] = b1.reshape(4, 128).T
    wmisc[0:64, WM_B2] = b2
    wmisc[:, WM_BNG:WM_BNG + 4] = bn_g.reshape(4, 128).T
    wmisc[:, WM_BNB:WM_BNB + 4] = bn_b.reshape(4, 128).T
    wmisc[0:64, WM_SEL:WM_SEL + 8] = np.tile(np.eye(8, dtype=f), (8, 1))

    wpk16 = np.zeros((128, W16_N), f)
    w1t = w1.T.astype(f)
    for cc in range(4):
        wpk16[:, W16_W1 + 512 * cc:W16_W1 + 512 * (cc + 1)] = \
            w1t[128 * cc:128 * (cc + 1), :]
    w2t = w2.T.astype(f)
    for oc in range(4):
        wpk16[:, W16_W2 + 64 * oc:W16_W2 + 64 * (oc + 1)] = \
            w2t[128 * oc:128 * (oc + 1), :]
    wv_ext = np.concatenate([wv, bv[:, None]], axis=1)  # [512, 65]
    for j in range(4):
        wpk16[:, W16_WVT + 65 * j:W16_WVT + 65 * (j + 1)] = \
            wv_ext[128 * j:128 * (j + 1), :]
    wpk16 = _to_bf16(wpk16)

    shared = {"wqek": wqek, "wmisc": wmisc, "wpk16": wpk16}
    in_maps = []
    for b in range(B):
        m = dict(shared)
        for key, src in (("qt", q), ("kt", k)):
            ext = np.concatenate([src[b].reshape(64, HW), ones], axis=0)
            # pre-tiled transpose: [p, 65*j+ci] = ext[ci, 128j+p]
            m[key] = _to_bf16(
                ext.T.reshape(32, 128, 65).transpose(1, 0, 2).reshape(128, -1))
        m["v"] = _to_bf16(np.concatenate([v[b].reshape(64, HW), ones], axis=0))
        in_maps.append(m)
    res = bass_utils.run_bass_kernel_spmd(
        nc, in_maps, core_ids=list(range(8)), trace=trace, tmpdir=tmpdir)
    out = np.stack([res.results[b]["y"].reshape(C, 64, 64) for b in range(B)])
    return out.astype(np.float32), res


def kernel(q, k, v, wq, bq, wk, bk, wv, bv, bn_g, bn_b, w1, b1, w2, b2):
    out, _ = _run(q, k, v, wq, bq, wk, bk, wv, bv, bn_g, bn_b, w1, b1, w2, b2)
    return out
